# revision 16
# baseline (speedup 1.0000x reference)
"""ConflateLoss Trainium2 kernel.

loss = -sum_b log_softmax(10 * cos_sim(anchor_b, match[cand_idx_b]))[0] / ln(10)
with B=16384, D=128, 50 candidates per anchor (slot 0 = positive b, 1..49 = neg_idx).

Strategy (8 NeuronCores, data-parallel over B):
  Instead of gathering candidate ROWS (102400 x 256B SWDGE descriptors per core,
  ~0.9us each), compute the DENSE similarity block and gather SCALARS on-chip:

  Phase 0 (each core): normalize this core's 2048-row match shard -> bf16
    DRAM, AllGather the normalized shards to the full [16384, 128] bf16 table
    (8x less host->device traffic than replicating match); normalize this
    core's anchors by 10/||a|| (gamma folded) -> bf16 DRAM; DMA-transpose both
    into SBUF as mhatT [128d, 16384] and ahatT [128d, 2048].
  Phase 1: per 128-anchor block k: TensorE computes sims = ahatT_k^T @ mhatT
    -> [128 anchors, 16384 j] f32 in PSUM (4-bank chunks), ACT/DVE copy to
    SBUF. gpsimd ap_gather pulls 800 scalars per 16-partition group (16
    anchors x 50 candidates, shared-index semantics) -> [128, 800]. ACT exp,
    DVE segment-reduce [128,16,50]->[128,16], one-hot lane mask picks each
    anchor's own sum -> softmax denominator; same mask on slot-0 logits gives
    logit0. loss_k = ln(den) - logit0.
  Output: [128, 16] per-core partial losses; host sums and divides by ln(10).
"""

import math

import numpy as np

import concourse.bacc as bacc
import concourse.bass as bass
import concourse.tile as tile
from concourse import mybir
from concourse.bass_utils import run_bass_kernel_spmd

B = 16384
D = 128
N_NEG = 49
N_CAND = 50
N_CORES = 8
B_SHARD = B // N_CORES  # 2048 anchors per core
N_BLK = B_SHARD // 128  # 16 blocks of 128 anchors
N_GTOK = 16 * N_CAND  # 800 gathered scalars per 16-partition group
IDX_COLS = N_GTOK // 16  # 50 int16 index columns per partition
M_GROUPS = 2  # local match shard normalized in 2 groups of 1024
G_ROWS = B_SHARD // M_GROUPS // 128  # 8 rows per partition per group
MM_N = 512  # moving columns per matmul (1 PSUM bank)
N_CHUNK = B // MM_N  # 8 matmul chunks per block

F32 = mybir.dt.float32
BF16 = mybir.dt.bfloat16
AF = mybir.ActivationFunctionType


def build_bass() -> bacc.Bacc:
    nc = bacc.Bacc("TRN2", debug=False, num_devices=N_CORES)

    match_in = nc.dram_tensor("match_shard", [B_SHARD, D], BF16,
                              kind="ExternalInput")
    anchors_in = nc.dram_tensor("anchors", [B_SHARD, D], BF16,
                                kind="ExternalInput")
    idx_in = nc.dram_tensor("idx", [128, N_BLK, IDX_COLS], mybir.dt.int16,
                            kind="ExternalInput")
    mask_in = nc.dram_tensor("mask", [128, 16], F32, kind="ExternalInput")
    out = nc.dram_tensor("out", [128, N_BLK], F32, kind="ExternalOutput")

    with tile.TileContext(nc) as tc:
        with (
            tc.tile_pool(name="dram", bufs=1, space="DRAM") as dram_pool,
            tc.tile_pool(name="persist", bufs=1) as persist,
        ):
            mhat_shard = dram_pool.tile([B_SHARD, D], BF16)
            mhat = dram_pool.tile([B, D], BF16, addr_space="Shared")
            ahat = dram_pool.tile([B_SHARD, D], BF16)
            mhatT = persist.tile([128, B], BF16)
            ahatT = persist.tile([128, B_SHARD], BF16)
            idxs = persist.tile([128, N_BLK, IDX_COLS], mybir.dt.int16)
            mask = persist.tile([128, 16], F32)
            lossacc = persist.tile([128, N_BLK], F32)

            nc.sync.dma_start(out=idxs, in_=idx_in.ap())
            nc.sync.dma_start(out=mask, in_=mask_in.ap())

            # ---- Phase 0a: normalize local match shard -> bf16 unit rows
            match_r = match_in.ap().rearrange(
                "(g p t) d -> g p t d", p=128, t=G_ROWS)
            mhat_r = mhat_shard[:].rearrange(
                "(g p t) d -> g p t d", p=128, t=G_ROWS)
            with (
                tc.tile_pool(name="mload", bufs=2) as mload,
                tc.tile_pool(name="msq", bufs=2) as msq,
                tc.tile_pool(name="mnorm", bufs=3) as mnorm,
                tc.tile_pool(name="mhatsb", bufs=2) as mhatsb,
            ):
                for g in range(M_GROUPS):
                    mf = mload.tile([128, G_ROWS, D], BF16)
                    nc.sync.dma_start(out=mf, in_=match_r[g])
                    sq = msq.tile([128, G_ROWS, D], F32)
                    nc.scalar.activation(out=sq, in_=mf, func=AF.Square)
                    nsq = mnorm.tile([128, G_ROWS], F32)
                    nc.vector.tensor_reduce(
                        out=nsq, in_=sq, axis=mybir.AxisListType.X,
                        op=mybir.AluOpType.add)
                    rin = mnorm.tile([128, G_ROWS], F32)
                    nc.vector.reciprocal(out=rin, in_=nsq)
                    inv = mnorm.tile([128, G_ROWS], F32)
                    nc.scalar.activation(out=inv, in_=rin, func=AF.Sqrt)
                    mh = mhatsb.tile([128, G_ROWS, D], BF16)
                    for t in range(G_ROWS):
                        nc.scalar.mul(out=mh[:, t, :], in_=mf[:, t, :],
                                      mul=inv[:, t:t + 1])
                    nc.sync.dma_start(out=mhat_r[g], in_=mh)

                # ---- Phase 0b: anchors scaled by 10/||a|| -> ahat bf16
                anch_r = anchors_in.ap().rearrange("(k p) d -> p k d", p=128)
                ahat_r = ahat[:].rearrange("(k p) d -> p k d", p=128)
                af = mload.tile([128, N_BLK, D], BF16, tag="af")
                nc.sync.dma_start(out=af, in_=anch_r)
                asq = msq.tile([128, N_BLK, D], F32, tag="asq")
                nc.scalar.activation(out=asq, in_=af, func=AF.Square)
                nsqa = mnorm.tile([128, N_BLK], F32, tag="nsqa")
                nc.vector.tensor_reduce(
                    out=nsqa, in_=asq, axis=mybir.AxisListType.X,
                    op=mybir.AluOpType.add)
                ra = mnorm.tile([128, N_BLK], F32, tag="ra")
                nc.vector.reciprocal(out=ra, in_=nsqa)
                sca = mnorm.tile([128, N_BLK], F32, tag="sca")
                # sqrt(100/nsq) = 10/||a||  (gamma=10 folded into the anchor)
                nc.scalar.activation(out=sca, in_=ra, func=AF.Sqrt, scale=100.0)
                ab = mhatsb.tile([128, N_BLK, D], BF16, tag="ab")
                for k in range(N_BLK):
                    nc.scalar.mul(out=ab[:, k, :], in_=af[:, k, :],
                                  mul=sca[:, k:k + 1])
                nc.sync.dma_start(out=ahat_r, in_=ab)

            # ---- Phase 0c: AllGather shards -> full normalized match table
            nc.gpsimd.collective_compute(
                "AllGather", mybir.AluOpType.bypass,
                replica_groups=[list(range(N_CORES))],
                ins=[mhat_shard[:]], outs=[mhat[:]])

            # ---- Phase 0d: DMA-transpose both into SBUF (d on partitions)
            nc.sync.dma_start(out=mhatT, in_=mhat[:], transpose=True)
            nc.sync.dma_start(out=ahatT, in_=ahat[:], transpose=True)

            # ---- Phase 1: dense sims + scalar gather + softmax per block
            with (
                tc.tile_pool(name="psum", bufs=8, space="PSUM") as psump,
                tc.tile_pool(name="sims", bufs=2) as simsp,
                tc.tile_pool(name="gath", bufs=2) as gathp,
                tc.tile_pool(name="small", bufs=4) as small,
            ):
                for k in range(N_BLK):
                    sims = simsp.tile([128, B], F32)
                    for c in range(N_CHUNK):
                        ps = psump.tile([128, MM_N], F32)
                        nc.tensor.matmul(
                            out=ps,
                            lhsT=ahatT[:, k * 128:(k + 1) * 128],
                            rhs=mhatT[:, c * MM_N:(c + 1) * MM_N],
                            start=True, stop=True)
                        dst = sims[:, c * MM_N:(c + 1) * MM_N]
                        if c % 2 == 0:
                            nc.scalar.copy(out=dst, in_=ps)
                        else:
                            nc.vector.tensor_copy(out=dst, in_=ps)
                    gath = gathp.tile([128, N_GTOK], F32)
                    nc.gpsimd.ap_gather(
                        out_ap=gath[:], in_ap=sims[:], idxs_ap=idxs[:, k, :],
                        channels=128, num_elems=B, d=1, num_idxs=N_GTOK)
                    e = gathp.tile([128, N_GTOK], F32, tag="e")
                    nc.scalar.activation(out=e, in_=gath, func=AF.Exp)
                    s16 = small.tile([128, 16], F32)
                    nc.vector.tensor_reduce(
                        out=s16, in_=e[:].rearrange("p (l n) -> p l n", n=N_CAND),
                        axis=mybir.AxisListType.X, op=mybir.AluOpType.add)
                    sm = small.tile([128, 16], F32)
                    nc.vector.tensor_tensor(
                        out=sm, in0=s16, in1=mask, op=mybir.AluOpType.mult)
                    den = small.tile([128, 1], F32)
                    nc.vector.tensor_reduce(
                        out=den, in_=sm, axis=mybir.AxisListType.X,
                        op=mybir.AluOpType.add)
                    # slot-0 logit of each lane, masked to the anchor's own lane
                    l0g = small.tile([128, 16], F32, tag="l0g")
                    nc.vector.tensor_tensor(
                        out=l0g,
                        in0=gath[:].rearrange(
                            "p (l n) -> p l n", n=N_CAND)[:, :, 0],
                        in1=mask, op=mybir.AluOpType.mult)
                    l0 = small.tile([128, 1], F32, tag="l0")
                    nc.vector.tensor_reduce(
                        out=l0, in_=l0g, axis=mybir.AxisListType.X,
                        op=mybir.AluOpType.add)
                    lden = small.tile([128, 1], F32, tag="lden")
                    nc.scalar.activation(out=lden, in_=den, func=AF.Ln)
                    nc.vector.tensor_tensor(
                        out=lossacc[:, k:k + 1], in0=lden, in1=l0,
                        op=mybir.AluOpType.subtract)

            nc.sync.dma_start(out=out.ap(), in_=lossacc)

    nc.compile()
    return nc


def make_in_maps(anchor_embedding, match_embedding, neg_idx):
    bf16 = mybir.dt.np(BF16)
    match = np.ascontiguousarray(np.asarray(match_embedding)).astype(bf16)
    anchors = np.ascontiguousarray(np.asarray(anchor_embedding)).astype(bf16)
    nidx = np.asarray(neg_idx).astype(np.int64)

    # one-hot lane mask: anchor on partition p owns lane p % 16 of its group
    mask = np.zeros((128, 16), np.float32)
    mask[np.arange(128), np.arange(128) % 16] = 1.0

    in_maps = []
    for c in range(N_CORES):
        lo = c * B_SHARD
        # [2048, 50] candidate j-indices: slot 0 = positive (global anchor id)
        cand_idx = np.concatenate(
            [np.arange(lo, lo + B_SHARD, dtype=np.int64)[:, None],
             nidx[lo:lo + B_SHARD]], axis=1).astype(np.int16)
        # per block k, group g: token i (= lane*50 + slot) stored at
        # partition 16g + i%16, column i//16  (ap_gather 16-lane wrap)
        idx_host = np.empty((128, N_BLK, IDX_COLS), np.int16)
        toks = cand_idx.reshape(N_BLK, 8, 16 * N_CAND)  # [k, g, lane*50+n]
        wrapped = toks.reshape(N_BLK, 8, IDX_COLS, 16).transpose(1, 3, 0, 2)
        idx_host[:] = wrapped.reshape(128, N_BLK, IDX_COLS)
        in_maps.append({
            "match_shard": match[lo:lo + B_SHARD],
            "anchors": anchors[lo:lo + B_SHARD],
            "idx": idx_host,
            "mask": mask,
        })
    return in_maps


_NC_CACHE = None


def kernel(anchor_embedding, match_embedding, neg_idx) -> np.ndarray:
    global _NC_CACHE
    if _NC_CACHE is None:
        _NC_CACHE = build_bass()
    nc = _NC_CACHE
    in_maps = make_in_maps(anchor_embedding, match_embedding, neg_idx)
    res = run_bass_kernel_spmd(nc, in_maps, core_ids=list(range(N_CORES)))
    total = sum(float(r["out"].astype(np.float64).sum()) for r in res.results)
    return np.asarray(total / math.log(10.0), dtype=np.float32)
